# revision 22
# baseline (speedup 1.0000x reference)
"""Trainium2 Bass kernel for nn_DeepQNetIVCML (GNN message passing).

Reference computation per (b, a) pair:
  multi-hop coverage over a sparse binary adjacency (3 steps), weighted
  feature aggregation, mask + mean-normalize, then a small shared MLP.

Sharding: 128 (b, a) pairs split across 8 cores (16 pairs each; every
core sees exactly one b). MLP weights are replicated.

Key kernel ideas (v2 — DMA-byte-minimized):
  - Propagation runs in "path count" space: p_{t+1} = A^T p_t with no
    thresholding between steps (support(p_t) is exact under any
    non-negative rounding), so cover_t = min(prefix_sum, 1) and the
    per-node weight is a telescoped linear combination of covers.
    Adjacency and seed vectors are binary -> exact in fp8.
  - fea = F^T w is computed with F as the *stationary* operand
    (24 [128,128] stationary tiles per pair, 1-column moving operand),
    which directly yields fea^T in column layout — no transposes.
    F ships as a single bf16 copy (~2^-9 relative error, far inside the
    2e-2 gate).  mask/denominator/ALPHA^4 fold into the per-pair scalar
    that scales w before the bf16 cast, so no per-partition scale vector
    is ever needed.
  - The whole MLP runs per pair in column layout with weight-stationary
    1-column matmuls; biases are folded in as rank-1 ([1,128] stationary
    x [1,1] ones) accumulation matmuls, so each stage needs exactly one
    Relu activation op over [128, DG].
  - All weights/q ship in bf16 (replicated per core).
"""

import os
import sys

for _p in ("/opt/trn_rl_repo", "/opt/pypackages"):
    if os.path.isdir(_p) and _p not in sys.path:
        sys.path.insert(0, _p)

import ml_dtypes
import numpy as np

import concourse.bacc as bacc
from concourse import masks
import concourse.mybir as mybir
from concourse.bass_utils import run_bass_kernel_spmd  # noqa: F401  (spmd path helper)
from concourse.tile import TileContext

B, A, N, D, L = 4, 32, 512, 768, 128
ALPHA = 0.8
STEP_NUM = 3
NCORES = 8
P_PER = (B * A) // NCORES  # pairs per core
NCH = N // 128             # node chunks
DG = D // 128              # feature chunks

BF16 = mybir.dt.bfloat16
F8 = mybir.dt.float8e4
F32 = mybir.dt.float32
BF16_NP = ml_dtypes.bfloat16
F8_NP = ml_dtypes.float8_e4m3

_PROG = None
LAST_RESULT = None


def _build():
    nc = bacc.Bacc("TRN2", target_bir_lowering=False, debug=False,
                   num_devices=NCORES)

    a_pre = nc.dram_tensor("a_pre", [128, P_PER * NCH * (N // 2)], F8,
                           kind="ExternalInput")
    f_pre = nc.dram_tensor("f_pre", [128, P_PER * NCH * D], BF16,
                           kind="ExternalInput")
    s0_pre = nc.dram_tensor("s0_pre", [128, P_PER * NCH], F8,
                            kind="ExternalInput")
    mask_pre = nc.dram_tensor("mask_pre", [1, P_PER], F32,
                              kind="ExternalInput")
    q_pre = nc.dram_tensor("q_pre", [L, D], BF16, kind="ExternalInput")
    w1_pre = nc.dram_tensor("w1_pre", [128, DG * D], BF16,
                            kind="ExternalInput")
    w2_pre = nc.dram_tensor("w2_pre", [128, 2 * DG * D], BF16,
                            kind="ExternalInput")
    w3_pre = nc.dram_tensor("w3_pre", [128, DG], BF16, kind="ExternalInput")
    b1r_pre = nc.dram_tensor("b1r_pre", [1, D], BF16, kind="ExternalInput")
    b2r_pre = nc.dram_tensor("b2r_pre", [1, D], BF16, kind="ExternalInput")
    b3_pre = nc.dram_tensor("b3_pre", [1, 1], F32, kind="ExternalInput")
    y_out = nc.dram_tensor("y", [P_PER, 1], F32, kind="ExternalOutput")

    mult = mybir.AluOpType.mult
    add = mybir.AluOpType.add
    relu = mybir.ActivationFunctionType.Relu

    # per-cover weights scaled by ALPHA^-4: exact dyadic rationals
    c_init = 1.0 / ALPHA**3 - 1.0 / ALPHA**2       # 0.390625
    coefs = [1.0 / ALPHA**2 - 1.0 / ALPHA,         # 0.3125
             1.0 / ALPHA - 1.0,                    # 0.25
             1.0]
    a4 = float(np.float32(ALPHA) ** 4)

    with TileContext(nc) as tc:
        with (
            tc.tile_pool(name="const", bufs=1) as cpool,
            tc.tile_pool(name="weights", bufs=1) as wpool,
            tc.tile_pool(name="abuf", bufs=8) as apool,
            tc.tile_pool(name="pbuf", bufs=16) as ppool,
            tc.tile_pool(name="fbuf", bufs=16) as fpool,
            tc.tile_pool(name="small", bufs=4) as spool,
            tc.tile_pool(name="hbuf", bufs=6) as hpool,
        ):
            ones128 = cpool.tile([128, 1], F32)
            nc.vector.memset(ones128[:], 1.0)
            onesL = cpool.tile([128, 1], BF16)
            nc.vector.memset(onesL[:], 1.0 / L)
            ones_row = cpool.tile([1, 128], F32)
            nc.vector.memset(ones_row[:], 1.0)
            ones16 = cpool.tile([1, P_PER], F32)
            nc.vector.memset(ones16[:], 1.0)
            ones16b = cpool.tile([1, P_PER], BF16)
            nc.vector.memset(ones16b[:], 1.0)
            one1b = cpool.tile([1, 1], BF16)
            nc.vector.memset(one1b[:], 1.0)

            # s0/mask first (tiny; pair-0 propagation needs them), then
            # weights on the ACT HWDGE ring: the pair stream rides the SP
            # ring, so the rings interleave at the DMA engines and the
            # weights are resident well before pair 0's MLP.
            s0_sb = cpool.tile([128, P_PER * NCH], F8)
            nc.scalar.dma_start(s0_sb[:], s0_pre[:])
            mask_sb = cpool.tile([1, P_PER], F32)
            nc.scalar.dma_start(mask_sb[:], mask_pre[:])

            # all A tiles stream FIRST so every pair's propagation (and the
            # A-side den/inv/ubf chain) completes long before the F stream
            # ends; F tiles follow and each pair's fea fires as its F lands.
            # Stream plan (one shared 360GB/s DMA-engine pool; three DGE
            # rings feed it so short transfers never leave it idle, and
            # neither the ACT nor DVE sequencer ever issues a DMA -- a DMA
            # instruction holds its ring's SEQ ~1.2us, which on the ACT ring
            # would delay the first unpack op to ~20us and stall the whole
            # propagation pipeline):
            #   ACT ring : s0, mask (tiny, first)
            #   SP ring  : P0..P15, F8..F15, w1, w2a   (w2a lands LAST: the
            #              post-stream tail is just h2 -> y -> out-DMA)
            #   Pool ring: F0..F7, q, w2b, b1r, b2r6, b3, w3 (SWDGE; fills
            #              the gaps the short P transfers leave early, and
            #              feeds q_block by ~30us)
            staged_a, staged_f = {}, {}
            HN = NCH * (N // 2)
            for p in range(P_PER):
                P_sb = ppool.tile([128, HN], F8, tag="P", name="P_sb")
                nc.sync.dma_start(P_sb[:], a_pre[:, p * HN:(p + 1) * HN])
                staged_a[p] = P_sb
            for p in range(P_PER):
                F_sb = fpool.tile([128, NCH * D], BF16, tag="F", name="F_sb")
                eng = nc.gpsimd if p < 8 else nc.sync
                eng.dma_start(F_sb[:],
                              f_pre[:, p * NCH * D:(p + 1) * NCH * D])
                staged_f[p] = F_sb
            q_sb = wpool.tile([L, D], BF16)
            nc.gpsimd.dma_start(q_sb[:], q_pre[:])
            w2_sb = wpool.tile([128, 2 * DG * D], BF16)
            nc.gpsimd.dma_start(w2_sb[:, DG * D:], w2_pre[:, DG * D:])
            b1r_sb = cpool.tile([1, D], BF16)
            nc.gpsimd.dma_start(b1r_sb[:], b1r_pre[:])
            b2r_sb = cpool.tile([1, D], BF16)
            nc.gpsimd.dma_start(b2r_sb[:], b2r_pre[:])
            b3_sb = cpool.tile([1, 1], F32)
            nc.gpsimd.dma_start(b3_sb[:], b3_pre[:])
            w3_sb = wpool.tile([128, DG], BF16)
            nc.gpsimd.dma_start(w3_sb[:], w3_pre[:])
            w1_sb = wpool.tile([128, DG * D], BF16)
            nc.sync.dma_start(w1_sb[:], w1_pre[:])
            nc.sync.dma_start(w2_sb[:, 0:DG * D], w2_pre[:, 0:DG * D])

            qT = cpool.tile([128, DG], BF16)
            qb2r_sb = cpool.tile([1, D], F32)
            ysb = cpool.tile([P_PER, 1], F32)
            nfT = wpool.tile([128, DG * P_PER], BF16)

            with tc.tile_pool(name="qps", bufs=1, space="PSUM") as q_psum:

                def q_block():
                    # q-side of the MLP, once per core: qT = (mean_L q)^T as
                    # bf16 columns; qb2 = W2_q^T qT in column form; then a PE
                    # transpose + b2 add gives qb2r6 [DG, 128] whose rows
                    # feed the rank-1 bias fold in the h2 stage.  Emitted
                    # after the fea phase so its weight-DMA waits never
                    # block time-critical PE work.
                    qps_t = q_psum.tile([128, DG], F32, tag="qt")
                    for g in range(DG):
                        nc.tensor.matmul(qps_t[:, g:g + 1],
                                         q_sb[:, g * 128:(g + 1) * 128],
                                         onesL[:], start=True, stop=True)
                    nc.scalar.copy(qT[:], qps_t[:])
                    qrow = q_psum.tile([1, D], F32, tag="qrow")
                    for lo, hi in ((0, 512), (512, D)):
                        for g in range(DG):
                            nc.tensor.matmul(
                                qrow[:, lo:hi], qT[:, g:g + 1],
                                w2_sb[:, (DG + g) * D + lo:(DG + g) * D + hi],
                                start=(g == 0), stop=False)
                        nc.tensor.matmul(qrow[:, lo:hi], one1b[:],
                                         b2r_sb[:, lo:hi],
                                         start=False, stop=True)
                    nc.scalar.copy(qb2r_sb[:], qrow[:])

                with (
                    tc.tile_pool(name="ppps", bufs=2, space="PSUM") as pp_psum,
                    tc.tile_pool(name="fdps", bufs=1, space="PSUM") as fd_psum,
                ):
                    # ---- phase 1: propagation + per-pair scalars (A only),
                    # two pairs interleaved so pair a's matmuls fill pair b's
                    # DVE bubbles ----
                    ubfs = {}

                    def emit_duo_tail(s, p):
                        # den = sum(cover) on one partition, then the folded
                        # scalar a4*mask/den broadcast to all 128 partitions
                        # via a rank-1 matmul, then folded into the bf16
                        # weight column.  Emitted one duo late so the PE
                        # stream never waits on the current duo's DVE chain.
                        dps = fd_psum.tile([1, 1], F32, tag="di", bufs=2,
                                           name="dps")
                        for c in range(NCH):
                            nc.tensor.matmul(dps[:], ones128[:],
                                             s["ct"][:, c:c + 1],
                                             start=(c == 0),
                                             stop=(c == NCH - 1))
                        den = spool.tile([1, 1], F32, tag="dens")
                        nc.vector.tensor_scalar_max(den[:], dps[:], 0.5)
                        rec = spool.tile([1, 1], F32, tag="rec")
                        nc.vector.reciprocal(rec[:], den[:])
                        inv = spool.tile([1, 1], F32, tag="inv")
                        nc.vector.scalar_tensor_tensor(
                            inv[:], rec[:], a4, mask_sb[:, p:p + 1],
                            op0=mult, op1=mult)
                        invp = fd_psum.tile([128, 1], F32, tag="di",
                                            bufs=2, name="invp")
                        nc.tensor.matmul(invp[:], ones_row[:], inv[:],
                                         start=True, stop=True)
                        ubf = spool.tile([128, NCH], BF16, tag="ubf",
                                         bufs=P_PER)
                        nc.vector.tensor_scalar_mul(ubf[:], s["wcol"][:],
                                                    invp[:])
                        ubfs[p] = ubf

                    pending = []
                    for pp in range(0, P_PER, 2):
                        duo = (pp, pp + 1)
                        st = {}
                        for p in duo:
                            s0c = s0_sb[:, p * NCH:(p + 1) * NCH]
                            pcur = spool.tile([128, NCH], F8, tag="pcur")
                            nc.vector.tensor_copy(pcur[:], s0c)
                            pref = spool.tile([128, NCH], F32, tag="pref")
                            nc.vector.tensor_copy(pref[:], s0c)
                            wcol = spool.tile([128, NCH], F32, tag="wcol")
                            nc.vector.tensor_scalar_mul(wcol[:], pref[:],
                                                        c_init)
                            ct = spool.tile([128, NCH], F32, tag="ct")
                            # unpack 2 adjacency entries per fp8 byte:
                            # v = a_even - 8*a_odd; a_even = relu(v) (exact
                            # {0,1}); a_odd = relu(-v) = {0,7,8} whose support
                            # equals a_odd -- valid because the propagation
                            # only ever uses supports (everything re-clamps).
                            P_sb = staged_a.pop(p)
                            A_sb = apool.tile([128, NCH * N], F8, tag="A",
                                              name="A_sb")
                            nc.scalar.activation(A_sb[:, 0::2], P_sb[:], relu)
                            nc.scalar.activation(A_sb[:, 1::2], P_sb[:], relu,
                                                 scale=-1.0)
                            st[p] = dict(A=A_sb, pcur=pcur,
                                         pref=pref, wcol=wcol, ct=ct)

                        for t in range(STEP_NUM):
                            for p in duo:
                                s = st[p]
                                ps = pp_psum.tile([128, NCH], F32, tag="pp")
                                s["ps"] = ps
                                for oc in range(NCH):
                                    base = oc * 128
                                    for ic in range(NCH):
                                        nc.tensor.matmul(
                                            ps[:, oc:oc + 1],
                                            s["A"][:, ic * N + base:
                                                   ic * N + base + 128],
                                            s["pcur"][:, ic:ic + 1],
                                            start=(ic == 0),
                                            stop=(ic == NCH - 1),
                                        )
                            for p in duo:
                                s = st[p]
                                ps = s["ps"]
                                # clamp to {0,1}: e4m3 overflows above 448
                                pnext = spool.tile([128, NCH], F8, tag="pcur")
                                nc.vector.tensor_scalar_min(pnext[:], ps[:],
                                                            1.0)
                                nc.vector.tensor_add(s["pref"][:],
                                                     s["pref"][:], ps[:])
                                nc.vector.tensor_scalar_min(s["ct"][:],
                                                            s["pref"][:], 1.0)
                                nc.vector.scalar_tensor_tensor(
                                    s["wcol"][:], s["ct"][:], coefs[t],
                                    s["wcol"][:], op0=mult, op1=add)
                                s["pcur"] = pnext

                        for pd, sd in pending:
                            emit_duo_tail(sd, pd)
                        pending = [(p, st[p]) for p in duo]

                    for pd, sd in pending:
                        emit_duo_tail(sd, pd)

                    # ---- phase 2: fea^T columns as each F tile lands ----
                    for p in range(P_PER):
                        F_sb = staged_f.pop(p)
                        ubf = ubfs.pop(p)
                        fps = fd_psum.tile([128, DG], F32, tag="fea", bufs=1)
                        for g in range(DG):
                            for c in range(NCH):
                                nc.tensor.matmul(
                                    fps[:, g:g + 1],
                                    F_sb[:, c * D + g * 128:
                                         c * D + g * 128 + 128],
                                    ubf[:, c:c + 1],
                                    start=(c == 0), stop=(c == NCH - 1))
                        nc.scalar.activation(nfT[:, p::P_PER], fps[:], relu)

                    q_block()

                with tc.tile_pool(name="mhps", bufs=2, space="PSUM") as mh_ps:
                    # ---- phase 3: batched MLP, biases folded as rank-1
                    # matmuls so each stage needs exactly one Relu ACT op ----
                    h1ps = mh_ps.tile([128, DG * P_PER], F32, tag="h")
                    for go in range(DG):
                        lo = go * P_PER
                        for g in range(DG):
                            nc.tensor.matmul(
                                h1ps[:, lo:lo + P_PER],
                                w1_sb[:, g * D + go * 128:
                                      g * D + go * 128 + 128],
                                nfT[:, g * P_PER:(g + 1) * P_PER],
                                start=(g == 0), stop=False)
                        nc.tensor.matmul(h1ps[:, lo:lo + P_PER],
                                         b1r_sb[:, go * 128:(go + 1) * 128],
                                         ones16b[:], start=False, stop=True)
                    h1T = wpool.tile([128, DG * P_PER], BF16)
                    nc.scalar.activation(h1T[:], h1ps[:], relu)

                    h2ps = mh_ps.tile([128, DG * P_PER], F32, tag="h")
                    for go in range(DG):
                        lo = go * P_PER
                        for g in range(DG):
                            nc.tensor.matmul(
                                h2ps[:, lo:lo + P_PER],
                                w2_sb[:, g * D + go * 128:
                                      g * D + go * 128 + 128],
                                h1T[:, g * P_PER:(g + 1) * P_PER],
                                start=(g == 0), stop=False)
                        nc.tensor.matmul(h2ps[:, lo:lo + P_PER],
                                         qb2r_sb[:, go * 128:(go + 1) * 128],
                                         ones16[:], start=False, stop=True)
                    h2T = wpool.tile([128, DG * P_PER], BF16)
                    nc.scalar.activation(h2T[:], h2ps[:], relu)

                    yp = mh_ps.tile([P_PER, 1], F32, tag="ytr", bufs=1)
                    for g in range(DG):
                        nc.tensor.matmul(yp[:],
                                         h2T[:, g * P_PER:(g + 1) * P_PER],
                                         w3_sb[:, g:g + 1],
                                         start=(g == 0), stop=False)
                    nc.tensor.matmul(yp[:], ones16[:], b3_sb[:],
                                     start=False, stop=True)
                    nc.scalar.copy(ysb[:], yp[:])
                    nc.sync.dma_start(y_out[:], ysb[:])

    nc.compile()
    return nc


def get_program():
    global _PROG
    if _PROG is None:
        _PROG = _build()
    return _PROG


def _pack_adj(a_loc):
    """[P_PER, N, N] binary -> [128, P_PER*NCH*(N/2)] fp8, two entries per
    byte: v = a_even - 8*a_odd (all of {0,1,-8,-7} exact in e4m3)."""
    ch = a_loc.reshape(P_PER, NCH, 128, N).transpose(2, 0, 1, 3)
    pairs = ch.reshape(128, P_PER, NCH, N // 2, 2)
    packed = pairs[..., 0] - 8.0 * pairs[..., 1]
    return np.ascontiguousarray(
        packed.reshape(128, P_PER * NCH * (N // 2))).astype(F8_NP)


def _prep_core(core, query_fea, a_nei, vec_nei, fea_emb, nei_mask,
               W1, b1, W2, b2, W3, b3):
    b = (core * P_PER) // A
    a0 = (core * P_PER) % A
    a_loc = a_nei[b, a0:a0 + P_PER]
    f_loc = fea_emb[b, a0:a0 + P_PER]
    s_loc = vec_nei[b, a0:a0 + P_PER]
    f_chunked = np.ascontiguousarray(
        f_loc.reshape(P_PER, NCH, 128, D).transpose(2, 0, 1, 3)
        .reshape(128, P_PER * NCH * D))
    return {
        "a_pre": _pack_adj(a_loc),
        "f_pre": f_chunked.astype(BF16_NP),
        "s0_pre": np.ascontiguousarray(
            s_loc.reshape(P_PER, NCH, 128).transpose(2, 0, 1)
            .reshape(128, P_PER * NCH)).astype(F8_NP),
        "mask_pre": nei_mask[b, a0:a0 + P_PER, 0].reshape(1, P_PER)
        .astype(np.float32),
        "q_pre": query_fea[b].astype(BF16_NP),
        "w1_pre": np.ascontiguousarray(
            W1.reshape(DG, 128, D).transpose(1, 0, 2).reshape(128, DG * D))
        .astype(BF16_NP),
        "w2_pre": np.ascontiguousarray(
            W2.reshape(2 * DG, 128, D).transpose(1, 0, 2)
            .reshape(128, 2 * DG * D)).astype(BF16_NP),
        "w3_pre": np.ascontiguousarray(
            W3[:, 0].reshape(DG, 128).transpose(1, 0)).astype(BF16_NP),
        "b1r_pre": b1.reshape(1, D).astype(BF16_NP),
        "b2r_pre": b2.reshape(1, D).astype(BF16_NP),
        "b3_pre": b3.reshape(1, 1).astype(np.float32),
    }


_EXEC = None


def _make_exec():
    """Replicates bass2jax.run_bass_via_pjrt's multi-core path, but caches
    the jitted executable so repeated calls (and timing loops) skip
    recompilation."""
    global _EXEC
    if _EXEC is not None:
        return _EXEC
    import jax
    from jax.experimental.shard_map import shard_map
    from jax.sharding import Mesh, PartitionSpec

    from concourse import mybir as _mybir
    from concourse.bass2jax import (_bass_exec_p, install_neuronx_cc_hook,
                                    partition_id_tensor)

    nc = get_program()
    install_neuronx_cc_hook()
    partition_name = (nc.partition_id_tensor.name
                      if nc.partition_id_tensor else None)
    in_names, out_names, out_avals, zero_outs = [], [], [], []
    for alloc in nc.m.functions[0].allocations:
        if not isinstance(alloc, _mybir.MemoryLocationSet):
            continue
        name = alloc.memorylocations[0].name
        if alloc.kind == "ExternalInput":
            if name != partition_name:
                in_names.append(name)
        elif alloc.kind == "ExternalOutput":
            shape = tuple(alloc.tensor_shape)
            dtype = _mybir.dt.np(alloc.dtype)
            out_names.append(name)
            out_avals.append(jax.core.ShapedArray(shape, dtype))
            zero_outs.append(np.zeros(shape, dtype))
    n_params = len(in_names)
    all_in_names = list(in_names) + list(out_names)
    if partition_name is not None:
        all_in_names.append(partition_name)

    def _body(*args):
        operands = list(args)
        if partition_name is not None:
            operands.append(partition_id_tensor())
        outs = _bass_exec_p.bind(
            *operands,
            out_avals=tuple(out_avals),
            in_names=tuple(all_in_names),
            out_names=tuple(out_names),
            lowering_input_output_aliases=(),
            sim_require_finite=True,
            sim_require_nnan=True,
            nc=nc,
        )
        return tuple(outs)

    devices = jax.devices()[:NCORES]
    mesh = Mesh(np.asarray(devices), ("core",))
    n_outs = len(out_names)
    sharded = jax.jit(
        shard_map(_body, mesh=mesh,
                  in_specs=(PartitionSpec("core"),) * (n_params + n_outs),
                  out_specs=(PartitionSpec("core"),) * n_outs,
                  check_rep=False),
        keep_unused=True,
    )
    _EXEC = (sharded, in_names, out_names, out_avals, zero_outs, mesh)
    return _EXEC


def run_sharded(in_maps, reps=1):
    """Execute on 8 cores; returns (per-core results, [wall_ns per rep])."""
    import time as _time

    import jax

    sharded, in_names, out_names, out_avals, zero_outs, mesh = _make_exec()
    from jax.sharding import NamedSharding, PartitionSpec
    shard = NamedSharding(mesh, PartitionSpec("core"))
    concat_in = [
        jax.device_put(
            np.concatenate([np.asarray(in_maps[c][n])
                            for c in range(NCORES)], axis=0), shard)
        for n in in_names
    ]
    concat_zeros = [
        jax.device_put(
            np.zeros((NCORES * z.shape[0], *z.shape[1:]), z.dtype), shard)
        for z in zero_outs
    ]
    args = concat_in + concat_zeros
    jax.block_until_ready(args)
    out_arrs = None
    times = []
    for _ in range(max(1, reps)):
        t0 = _time.perf_counter()
        out_arrs = sharded(*args)
        jax.block_until_ready(out_arrs)
        times.append((_time.perf_counter() - t0) * 1e9)
    results = [
        {
            name: np.asarray(out_arrs[i]).reshape(
                NCORES, *out_avals[i].shape)[c]
            for i, name in enumerate(out_names)
        }
        for c in range(NCORES)
    ]
    return results, times


def kernel(query_fea, a_nei, vec_nei, fea_emb, nei_mask,
           W1, b1, W2, b2, W3, b3, trace=False, reps=1):
    global LAST_RESULT
    args = [np.asarray(x) for x in (query_fea, a_nei, vec_nei, fea_emb,
                                    nei_mask, W1, b1, W2, b2, W3, b3)]
    in_maps = [_prep_core(c, *args) for c in range(NCORES)]
    results, times = run_sharded(in_maps, reps=reps)
    LAST_RESULT = {"times_ns": times}
    ys = [results[c]["y"].reshape(P_PER) for c in range(NCORES)]
    return np.concatenate(ys).reshape(B, A, 1).astype(np.float32)
